# revision 1
# baseline (speedup 1.0000x reference)
"""Trainium2 Bass kernel for nn_MixFusionFeedForward (self-contained).

Data-parallel over the 16 video clips (2 per NeuronCore). Per clip:
  x[720,512] --DMA cast+transpose--> xT bf16 [512,720]
  "tconv": phase-decomposed stride-3 transposed conv == matmul1 + fold fused:
     img[(a,b)][ch, m, n] = sum_{s<S_a, t<S_b} (x @ w1[:,ch,a+3s,b+3t])[m-s,n-t]
     as 21 shifted matmuls accumulating in PSUM (bf16 in, fp32 accum).
  norm: imgn = img * invnorm + b1*invnorm  (DVE, PSUM -> SBUF bf16)
  unfold: X2[(phase,s,ch,t)][i,j] = imgn[phase][ch, i+s, j+t]  (SBUF->SBUF DMA)
  dwconv: depthwise 3x3/5x5 = diagonal 32x32 matmuls, 16 PE sub-array tiles
     concurrent (tile_position packing), taps accumulate in PSUM.
  gelu(+conv bias) on ScalarE evacuating PSUM -> g bf16
  mm2: out[l,:] = g.T @ w2r + b2 (bf16 matmul, fp32 accum)
"""
import sys
if '/opt/trn_rl_repo' not in sys.path:
    sys.path.insert(0, '/opt/trn_rl_repo')

import numpy as np
import ml_dtypes

D = 512
HD = 1960
NCH = 40
KH = KW = 7
HOUT, WOUT = 20, 36
L = HOUT * WOUT
T = 8
B = 2
NCLIP = B * T
NCORE = 8
CPC = NCLIP // NCORE
PM, PN = 22, 38
PMN = PM * PN
S = (3, 2, 2)
IH, IW = 60, 108
PH = PW = 3
PLH, PLW = 24, 40
PLANE = PLH * PLW
GROWS = 1024
NCHUNK = 16
BF16 = ml_dtypes.bfloat16

TCONV_MMS = [(s, t, a) for s in range(3) for t in range(3) for a in range(3)
             if s < S[a]]
assert len(TCONV_MMS) == 21


def _sem_rows():
    rows = []
    for half in (0, 1):
        cnt = 0
        for a in range(3):
            for b in range(3):
                for s in range(S[a]):
                    for ch in range(20):
                        for t in range(S[b]):
                            ki, kj = a + 3 * s, b + 3 * t
                            chfull = half * 20 + ch
                            rows.append(dict(half=half, a=a, b=b, s=s, ch=ch,
                                             t=t, cdw=chfull * 49 + ki * 7 + kj))
                            cnt += 1
        assert cnt == 980
        rows.extend([None] * (GROWS - 980))
    return rows


def _sem_to_phys(row):
    q, p = divmod(row, 128)
    r, o = divmod(p, 32)
    return q * 128 + 32 * ((r + q) % 4) + o


def build_consts(w1, b1, w3, b3, w5, b5, w2, b2):
    rows = _sem_rows()
    w1r = w1.reshape(D, NCH, KH, KW)
    b1r = b1.reshape(NCH, KH, KW)

    w1f = np.zeros((128, len(TCONV_MMS) * 4 * 120), np.float32)
    for mi, (s, t, a) in enumerate(TCONV_MMS):
        tl = np.zeros((D, 120), np.float32)
        for b_ in range(3):
            if t >= S[b_]:
                continue
            for half in (0, 1):
                cols = 40 * b_ + 20 * half + np.arange(20)
                tl[:, cols] = w1r[:, half * 20:half * 20 + 20,
                                  a + 3 * s, b_ + 3 * t]
        for k in range(4):
            w1f[:, (mi * 4 + k) * 120:(mi * 4 + k + 1) * 120] = \
                tl[k * 128:(k + 1) * 128]
    w1f = w1f.astype(BF16)

    nr = np.zeros(3 * PM)
    ncv = np.zeros(3 * PN)
    for i in range(HOUT):
        nr[3 * i:3 * i + KH] += 1
    for j in range(WOUT):
        ncv[3 * j:3 * j + KW] += 1
    invn = np.zeros((3, 3, PM, PN), np.float32)
    for a in range(3):
        for b_ in range(3):
            r = 3 * np.arange(PM) + a
            c = 3 * np.arange(PN) + b_
            rv = (r >= PH) & (r <= IH + PH - 1)
            cv = (c >= PW) & (c <= IW + PW - 1)
            with np.errstate(divide='ignore'):
                iv = 1.0 / np.outer(nr[r], ncv[c])
            iv[~rv, :] = 0
            iv[:, ~cv] = 0
            invn[a, b_] = iv
    b1img = np.zeros((NCH, 3, 3, PM, PN), np.float32)
    for a in range(3):
        for b_ in range(3):
            for s in range(S[a]):
                for t in range(S[b_]):
                    v = b1r[:, a + 3 * s, b_ + 3 * t]
                    b1img[:, a, b_, s:s + HOUT, t:t + WOUT] += v[:, None, None]
    invn_sb = np.zeros((128, 3 * PMN), np.float32)
    b1n_sb = np.zeros((128, 3 * PMN), np.float32)
    for a in range(3):
        for b_ in range(3):
            for half in (0, 1):
                for ch in range(20):
                    p = 40 * b_ + 20 * half + ch
                    iv = invn[a, b_]
                    invn_sb[p, a * PMN:(a + 1) * PMN] = iv.ravel()
                    b1n_sb[p, a * PMN:(a + 1) * PMN] = \
                        (b1img[half * 20 + ch, a, b_] * iv).ravel()
    invn_sb = invn_sb.astype(BF16)
    b1n_sb = b1n_sb.astype(BF16)

    def dw_weight(row, du, dv, k, off):
        info = rows[row]
        if info is None:
            return 0.0
        w = w3[info['cdw'], 0] if k == 3 else w5[info['cdw'] - HD // 2, 0]
        return float(w[du + off, dv + off])

    dga = np.zeros((128, 8 * 9 * 32), np.float32)
    dgb = np.zeros((128, 8 * 25 * 32), np.float32)
    for q in range(8):
        for r in range(4):
            for o in range(32):
                rowa = q * 128 + 32 * r + o
                rowb = (8 + q) * 128 + 32 * r + o
                for uvi, (du, dv) in enumerate(
                        (du, dv) for du in (-1, 0, 1) for dv in (-1, 0, 1)):
                    dga[32 * r + o, (q * 9 + uvi) * 32 + o] = \
                        dw_weight(rowa, du, dv, 3, 1)
                for uvi, (du, dv) in enumerate(
                        (du, dv) for du in (-2, -1, 0, 1, 2)
                        for dv in (-2, -1, 0, 1, 2)):
                    dgb[32 * r + o, (q * 25 + uvi) * 32 + o] = \
                        dw_weight(rowb, du, dv, 5, 2)
    dga = dga.astype(BF16)
    dgb = dgb.astype(BF16)

    w2r = np.zeros((128, NCHUNK * D), np.float32)
    bconv = np.zeros((128, NCHUNK), np.float32)
    for row in range(2048):
        info = rows[row]
        phys = _sem_to_phys(row)
        q, p = divmod(phys, 128)
        if info is not None:
            w2r[p, q * D:(q + 1) * D] = w2[info['cdw']]
            cdw = info['cdw']
            bconv[p, q] = b3[cdw] if cdw < HD // 2 else b5[cdw - HD // 2]
    w2r = w2r.astype(BF16)
    b2rep = np.tile(b2[None, :], (128, 1)).astype(np.float32)

    return dict(w1f=w1f, invn=invn_sb, b1n=b1n_sb, dga=dga, dgb=dgb,
                w2r=w2r, bconv=bconv, b2rep=b2rep)


def _unfold_plan():
    plan = []
    for half in (0, 1):
        base = half * GROWS
        cnt = 0
        for a in range(3):
            for b in range(3):
                for s in range(S[a]):
                    plan.append((half, a, b, s, base + cnt, 20 * S[b]))
                    cnt += 20 * S[b]
    return plan


_UNFOLD_PLAN = _unfold_plan()

_BUILT = None


def _build():
    global _BUILT
    if _BUILT is not None:
        return _BUILT
    import concourse.bacc as bacc
    import concourse.tile as tile
    import concourse.mybir as mybir
    import bass_rust
    from contextlib import ExitStack

    dt = mybir.dt
    AF = mybir.ActivationFunctionType
    OP = mybir.AluOpType

    def view(ap2d, p0, pcnt, off, dims, pstep=1):
        """Arbitrary free-dim view of a [128, F] tile, partitions
        p0, p0+pstep, ... (pcnt of them)."""
        tp = ap2d[p0:p0 + 1, :]
        pitch = ap2d.ap[0][0]
        return bass_rust.AP(tp.tensor, tp.offset + off,
                            [[pitch * pstep, pcnt]]
                            + [[s, c] for s, c in dims])

    nc = bacc.Bacc("TRN2", target_bir_lowering=False, debug=False,
                   enable_asserts=False, num_devices=NCORE)

    x_d = nc.dram_tensor("x_in", [CPC * L, D], dt.float32,
                         kind="ExternalInput").ap()
    w1f_d = nc.dram_tensor("w1f", [128, 21 * 4 * 120], dt.bfloat16,
                           kind="ExternalInput").ap()
    invn_d = nc.dram_tensor("invn", [128, 3 * PMN], dt.bfloat16,
                            kind="ExternalInput").ap()
    b1n_d = nc.dram_tensor("b1n", [128, 3 * PMN], dt.bfloat16,
                           kind="ExternalInput").ap()
    dga_d = nc.dram_tensor("dga", [128, 8 * 9 * 32], dt.bfloat16,
                           kind="ExternalInput").ap()
    dgb_d = nc.dram_tensor("dgb", [128, 8 * 25 * 32], dt.bfloat16,
                           kind="ExternalInput").ap()
    w2r_d = nc.dram_tensor("w2r", [128, NCHUNK * D], dt.bfloat16,
                           kind="ExternalInput").ap()
    bconv_d = nc.dram_tensor("bconv", [128, NCHUNK], dt.float32,
                             kind="ExternalInput").ap()
    b2rep_d = nc.dram_tensor("b2rep", [128, D], dt.float32,
                             kind="ExternalInput").ap()
    out_d = nc.dram_tensor("y_out", [CPC * L, D], dt.float32,
                           kind="ExternalOutput").ap()

    with tile.TileContext(nc) as tc, ExitStack() as ctx:
        dram_pool = ctx.enter_context(
            tc.tile_pool(name="dram", bufs=1, space="DRAM"))
        xbf_d = dram_pool.tile([CPC * L, D], dt.bfloat16)

        consts = ctx.enter_context(tc.tile_pool(name="consts", bufs=1))

        def cload(nm, dram_ap, shape, dtype):
            t = consts.tile(shape, dtype, tag=nm, name=f"c_{nm}")
            nc.sync.dma_start(t[:, :], dram_ap[:, :])
            return t

        w1f = cload('w1f', w1f_d, [128, 21 * 4 * 120], dt.bfloat16)
        invn = cload('invn', invn_d, [128, 3 * PMN], dt.bfloat16)
        b1n = cload('b1n', b1n_d, [128, 3 * PMN], dt.bfloat16)
        dga = cload('dga', dga_d, [128, 8 * 9 * 32], dt.bfloat16)
        dgb = cload('dgb', dgb_d, [128, 8 * 25 * 32], dt.bfloat16)
        w2r = cload('w2r', w2r_d, [128, NCHUNK * D], dt.bfloat16)
        bconv = cload('bconv', bconv_d, [128, NCHUNK], dt.float32)
        b2rep = cload('b2rep', b2rep_d, [128, D], dt.float32)

        nc.gpsimd.dma_start(xbf_d[:, :], x_d[:, :])

        xt_pool = ctx.enter_context(tc.tile_pool(name="xt", bufs=2))
        imgn_pool = ctx.enter_context(tc.tile_pool(name="imgn", bufs=2))
        x2_pool = ctx.enter_context(tc.tile_pool(name="x2", bufs=2))
        g_pool = ctx.enter_context(tc.tile_pool(name="g", bufs=1))
        osb_pool = ctx.enter_context(tc.tile_pool(name="osb", bufs=2))

        for clip in range(CPC):
            # ---------- xT ----------
            xt = xt_pool.tile([128, 4 * L], dt.bfloat16, tag="xt")
            for kc in range(4):
                nc.sync.dma_start(
                    xt[:, kc * L:(kc + 1) * L],
                    xbf_d[clip * L:(clip + 1) * L, kc * 128:(kc + 1) * 128],
                    transpose=True)

            # ---------- tconv ----------
            imgn = imgn_pool.tile([128, 3 * PMN], dt.bfloat16, tag="imgn")
            with tc.tile_pool(name="tcps", bufs=2, space="PSUM") as tps:
                for jh in range(2):
                    ps = [tps.tile([128, 512], dt.float32, tag=f"tc{a}",
                                   name=f"tcps{a}")
                          for a in range(3)]
                    nmm = {a: sum(1 for (_, _, aa) in TCONV_MMS if aa == a)
                           for a in range(3)}
                    cnt = {0: 0, 1: 0, 2: 0}
                    for mi, (s, t, a) in enumerate(TCONV_MMS):
                        cnt[a] += 1
                        if jh == 0:
                            wjd, loc_off, src_off = 19 - t, s * 19 + t, 0
                        else:
                            wjd, loc_off, src_off = 17 + t, s * 19, 19 - t
                        out_ap = view(ps[a], 0, 120, loc_off,
                                      [(19, HOUT), (1, wjd)])
                        for k in range(4):
                            rhs = view(xt, 0, 128, k * L + src_off,
                                       [(WOUT, HOUT), (1, wjd)])
                            lhsT = w1f[:, (mi * 4 + k) * 120:
                                       (mi * 4 + k + 1) * 120]
                            nc.tensor.matmul(out_ap, lhsT, rhs,
                                             start=(cnt[a] == 1 and k == 0),
                                             stop=(cnt[a] == nmm[a]
                                                   and k == 3))
                    for a in range(3):
                        dims = [(19, PM), (1, 19)]
                        ps_ap = view(ps[a], 0, 120, 0, dims)
                        im_ap = view(imgn, 0, 120, a * PMN + jh * 19,
                                     [(PN, PM), (1, 19)])
                        iv_ap = view(invn, 0, 120, a * PMN + jh * 19,
                                     [(PN, PM), (1, 19)])
                        b1_ap = view(b1n, 0, 120, a * PMN + jh * 19,
                                     [(PN, PM), (1, 19)])
                        nc.vector.tensor_tensor(im_ap, ps_ap, iv_ap, OP.mult)
                        nc.vector.tensor_tensor(im_ap, im_ap, b1_ap, OP.add)

            # ---------- unfold ----------
            x2a = x2_pool.tile([128, 8 * PLANE], dt.bfloat16, tag="x2a")
            x2b = x2_pool.tile([128, 8 * PLANE], dt.bfloat16, tag="x2b")
            for x2t in (x2a, x2b):
                nc.vector.memset(
                    view(x2t, 0, 128, 0, [(PLANE, 8), (1, 2 * PLW + 2)]), 0)
                nc.vector.memset(
                    view(x2t, 0, 128, 22 * PLW - 2,
                         [(PLANE, 8), (1, 2 * PLW + 2)]), 0)
                nc.vector.memset(
                    view(x2t, 0, 128, 2 * PLW + 38,
                         [(PLANE, 8), (PLW, 20), (1, 4)]), 0)
                # pad rows 980..1023 (chunk 7, partitions 84..127): zero the
                # whole plane so zero-diag matmuls see 0, not NaN garbage.
                # (32-aligned partition base required; unfold rewrites the
                # real rows 960..979 afterwards.)
                nc.vector.memset(x2t[64:128, 7 * PLANE:8 * PLANE], 0)

            for (half, a, b, s, r0g, n) in _UNFOLD_PLAN:
                sb = S[b]
                x2t = x2a if half == 0 else x2b
                pbase = 40 * b + 20 * half
                r0 = r0g % GROWS
                for t in range(sb):
                    # rows r0 + ch*sb + t for ch in 0..19; split where the
                    # 128-row chunk index changes
                    ch0 = 0
                    while ch0 < 20:
                        q = (r0 + t + ch0 * sb) // 128
                        ch1 = ch0
                        while ch1 < 20 and (r0 + t + ch1 * sb) // 128 == q:
                            ch1 += 1
                        nch_ = ch1 - ch0
                        p0 = (r0 + t + ch0 * sb) - q * 128
                        ssrc = view(imgn, pbase + ch0, nch_,
                                    a * PMN + s * PN + t,
                                    [(PN, HOUT), (1, WOUT)])
                        dst = view(x2t, p0, nch_,
                                   q * PLANE + 2 * PLW + 2,
                                   [(PLW, HOUT), (1, WOUT)], pstep=sb)
                        nc.sync.dma_start(dst, ssrc)
                        ch0 = ch1

            # ---------- dwconv ----------
            g = g_pool.tile([128, NCHUNK * L], dt.bfloat16, tag="g")
            with tc.tile_pool(name="dwps", bufs=2, space="PSUM") as dps:
                for grp in range(2):
                    x2t = x2a if grp == 0 else x2b
                    dg = dga if grp == 0 else dgb
                    nuv = 9 if grp == 0 else 25
                    ko = 1 if grp == 0 else 2
                    uvs = [(du, dv) for du in range(-ko, ko + 1)
                           for dv in range(-ko, ko + 1)]
                    for pg in range(2):
                        for jh in range(2):
                            ps = dps.tile([128, 4 * 512], dt.float32,
                                          tag="dw")
                            for uvi, (du, dv) in enumerate(uvs):
                                for ql in range(4):
                                    q = 4 * pg + ql
                                    for r in range(4):
                                        c = (r + q + 8 * grp) % 4
                                        lhsT = dg[32 * r:32 * r + 32,
                                                  (q * nuv + uvi) * 32:
                                                  (q * nuv + uvi + 1) * 32]
                                        rhs = view(
                                            x2t, 32 * r, 32,
                                            q * PLANE + (2 + du) * PLW
                                            + 2 + dv + jh * 18,
                                            [(PLW, HOUT), (1, 18)])
                                        out = view(ps, 32 * c, 32, ql * 512,
                                                   [(18, HOUT), (1, 18)])
                                        nc.tensor.matmul(
                                            out, lhsT, rhs,
                                            start=(uvi == 0),
                                            stop=(uvi == nuv - 1),
                                            tile_position=(32 * r, 32 * c))
                            for ql in range(4):
                                gq = 8 * grp + 4 * pg + ql
                                g_ap = view(g, 0, 128, gq * L + jh * 18,
                                            [(WOUT, HOUT), (1, 18)])
                                ps_ap = view(ps, 0, 128, ql * 512,
                                             [(18, HOUT), (1, 18)])
                                nc.scalar.activation(
                                    g_ap, ps_ap, AF.Gelu,
                                    bias=bconv[:, gq:gq + 1], scale=1.0)

            # ---------- mm2 ----------
            with tc.tile_pool(name="mmps", bufs=2, space="PSUM") as mps:
                for mt in range(6):
                    pso = mps.tile([128, D], dt.float32, tag="mm2")
                    for kc in range(NCHUNK):
                        lhsT = g[:, kc * L + mt * 120:kc * L + mt * 120 + 120]
                        rhs = w2r[:, kc * D:(kc + 1) * D]
                        nc.tensor.matmul(pso[0:120, :], lhsT, rhs,
                                         start=(kc == 0),
                                         stop=(kc == NCHUNK - 1))
                    osb = osb_pool.tile([128, D], dt.float32, tag="osb")
                    nc.vector.tensor_tensor(osb[0:120, :], pso[0:120, :],
                                            b2rep[0:120, :], OP.add)
                    nc.sync.dma_start(
                        out_d[clip * L + mt * 120:
                              clip * L + mt * 120 + 120, :],
                        osb[0:120, :])

    nc.compile()
    _BUILT = nc
    return nc


def kernel(**inputs):
    x = np.asarray(inputs['x'], np.float32)
    consts = build_consts(
        np.asarray(inputs['w1'], np.float32),
        np.asarray(inputs['b1'], np.float32),
        np.asarray(inputs['w3'], np.float32),
        np.asarray(inputs['b3'], np.float32),
        np.asarray(inputs['w5'], np.float32),
        np.asarray(inputs['b5'], np.float32),
        np.asarray(inputs['w2'], np.float32),
        np.asarray(inputs['b2'], np.float32))
    nc = _build()
    from concourse.bass_utils import run_bass_kernel_spmd

    xf = x.reshape(NCLIP, L, D)
    in_maps = []
    for core in range(NCORE):
        m = {k: consts[k] for k in consts}
        m['x_in'] = np.ascontiguousarray(
            xf[core * CPC:(core + 1) * CPC].reshape(CPC * L, D))
        in_maps.append(m)
    res = run_bass_kernel_spmd(nc, in_maps, core_ids=list(range(NCORE)))
    out = np.zeros((NCLIP, L, D), np.float32)
    for core in range(NCORE):
        out[core * CPC:(core + 1) * CPC] = \
            res.results[core]['y_out'].reshape(CPC, L, D)
    return out.reshape(B, T * L, D)



# revision 2
# speedup vs baseline: 1.0349x; 1.0349x over previous
"""Trainium2 Bass kernel for nn_MixFusionFeedForward (self-contained).

Data-parallel over the 16 video clips (2 per NeuronCore). Per clip:
  x[720,512] --DMA cast+transpose--> xT bf16 [512,720]
  "tconv": phase-decomposed stride-3 transposed conv == matmul1 + fold fused:
     img[(a,b)][ch, m, n] = sum_{s<S_a, t<S_b} (x @ w1[:,ch,a+3s,b+3t])[m-s,n-t]
     as 21 shifted matmuls accumulating in PSUM (bf16 in, fp32 accum).
  norm: imgn = img * invnorm + b1*invnorm  (DVE, PSUM -> SBUF bf16)
  unfold: X2[(phase,s,ch,t)][i,j] = imgn[phase][ch, i+s, j+t]  (SBUF->SBUF DMA)
  dwconv: depthwise 3x3/5x5 = diagonal 32x32 matmuls, 16 PE sub-array tiles
     concurrent (tile_position packing), taps accumulate in PSUM.
  gelu(+conv bias) on ScalarE evacuating PSUM -> g bf16
  mm2: out[l,:] = g.T @ w2r + b2 (bf16 matmul, fp32 accum)

v2 scheduling: both clips' input DMAs issued up front; tconv(0), tconv(1)
run back-to-back on PE while unfold(clip) DMAs (issued right after each
tconv) overlap the other clip's matmuls; then dw(0), mm2(0), dw(1), mm2(1).
Weight tiles padded to 128 columns so the compiler's fast-weight-load path
(FWL, needs NumWeights==128 and non-fp32) engages for tconv and mm2.
"""
import sys
if '/opt/trn_rl_repo' not in sys.path:
    sys.path.insert(0, '/opt/trn_rl_repo')

import numpy as np
import ml_dtypes

D = 512
HD = 1960
NCH = 40
KH = KW = 7
HOUT, WOUT = 20, 36
L = HOUT * WOUT
T = 8
B = 2
NCLIP = B * T
NCORE = 8
CPC = NCLIP // NCORE
PM, PN = 22, 38
PNP = 40                  # stored imgn row pitch (cols 38,39 zeroed)
PMN = PM * PNP
S = (3, 2, 2)
IH, IW = 60, 108
PH = PW = 3
PLH, PLW = 24, 40
PLANE = PLH * PLW
GROWS = 1024
NCHUNK = 16
GPAD = 8
BF16 = ml_dtypes.bfloat16

TCONV_MMS = [(s, t, a) for s in range(3) for t in range(3) for a in range(3)
             if s < S[a]]
assert len(TCONV_MMS) == 21


def _sem_rows():
    rows = []
    for half in (0, 1):
        cnt = 0
        for a in range(3):
            for b in range(3):
                for s in range(S[a]):
                    for ch in range(20):
                        for t in range(S[b]):
                            ki, kj = a + 3 * s, b + 3 * t
                            chfull = half * 20 + ch
                            rows.append(dict(half=half, a=a, b=b, s=s, ch=ch,
                                             t=t, cdw=chfull * 49 + ki * 7 + kj))
                            cnt += 1
        assert cnt == 980
        rows.extend([None] * (GROWS - 980))
    return rows


def _sem_to_phys(row):
    q, p = divmod(row, 128)
    r, o = divmod(p, 32)
    return q * 128 + 32 * ((r + q) % 4) + o


def build_consts(w1, b1, w3, b3, w5, b5, w2, b2):
    rows = _sem_rows()
    w1r = w1.reshape(D, NCH, KH, KW)
    b1r = b1.reshape(NCH, KH, KW)

    # tconv weights, padded to 128 columns per (mi, k) slice for FWL
    w1f = np.zeros((128, len(TCONV_MMS) * 4 * 128), np.float32)
    for mi, (s, t, a) in enumerate(TCONV_MMS):
        tl = np.zeros((D, 128), np.float32)
        for b_ in range(3):
            if t >= S[b_]:
                continue
            for half in (0, 1):
                cols = 40 * b_ + 20 * half + np.arange(20)
                tl[:, cols] = w1r[:, half * 20:half * 20 + 20,
                                  a + 3 * s, b_ + 3 * t]
        for k in range(4):
            w1f[:, (mi * 4 + k) * 128:(mi * 4 + k) * 128 + 128] = \
                tl[k * 128:(k + 1) * 128]
    w1f = w1f.astype(BF16)

    nr = np.zeros(3 * PM)
    ncv = np.zeros(3 * PN)
    for i in range(HOUT):
        nr[3 * i:3 * i + KH] += 1
    for j in range(WOUT):
        ncv[3 * j:3 * j + KW] += 1
    invn = np.zeros((3, 3, PM, PN), np.float32)
    for a in range(3):
        for b_ in range(3):
            r = 3 * np.arange(PM) + a
            c = 3 * np.arange(PN) + b_
            rv = (r >= PH) & (r <= IH + PH - 1)
            cv = (c >= PW) & (c <= IW + PW - 1)
            with np.errstate(divide='ignore'):
                iv = 1.0 / np.outer(nr[r], ncv[c])
            iv[~rv, :] = 0
            iv[:, ~cv] = 0
            invn[a, b_] = iv
    b1img = np.zeros((NCH, 3, 3, PM, PN), np.float32)
    for a in range(3):
        for b_ in range(3):
            for s in range(S[a]):
                for t in range(S[b_]):
                    v = b1r[:, a + 3 * s, b_ + 3 * t]
                    b1img[:, a, b_, s:s + HOUT, t:t + WOUT] += v[:, None, None]
    invn_sb = np.zeros((128, 3 * PMN), np.float32)
    b1n_sb = np.zeros((128, 3 * PMN), np.float32)
    for a in range(3):
        for b_ in range(3):
            for half in (0, 1):
                for ch in range(20):
                    p = 40 * b_ + 20 * half + ch
                    iv = np.zeros((PM, PNP), np.float32)
                    iv[:, :PN] = invn[a, b_]
                    bv = np.zeros((PM, PNP), np.float32)
                    bv[:, :PN] = b1img[half * 20 + ch, a, b_] * invn[a, b_]
                    invn_sb[p, a * PMN:(a + 1) * PMN] = iv.ravel()
                    b1n_sb[p, a * PMN:(a + 1) * PMN] = bv.ravel()
    invn_sb = invn_sb.astype(BF16)
    b1n_sb = b1n_sb.astype(BF16)

    def dw_weight(row, du, dv, k, off):
        info = rows[row]
        if info is None:
            return 0.0
        w = w3[info['cdw'], 0] if k == 3 else w5[info['cdw'] - HD // 2, 0]
        return float(w[du + off, dv + off])

    dga = np.zeros((128, 8 * 9 * 32), np.float32)
    dgb = np.zeros((128, 8 * 25 * 32), np.float32)
    for q in range(8):
        for r in range(4):
            for o in range(32):
                rowa = q * 128 + 32 * r + o
                rowb = (8 + q) * 128 + 32 * r + o
                for uvi, (du, dv) in enumerate(
                        (du, dv) for du in (-1, 0, 1) for dv in (-1, 0, 1)):
                    dga[32 * r + o, (q * 9 + uvi) * 32 + o] = \
                        dw_weight(rowa, du, dv, 3, 1)
                for uvi, (du, dv) in enumerate(
                        (du, dv) for du in (-2, -1, 0, 1, 2)
                        for dv in (-2, -1, 0, 1, 2)):
                    dgb[32 * r + o, (q * 25 + uvi) * 32 + o] = \
                        dw_weight(rowb, du, dv, 5, 2)
    dga = dga.astype(BF16)
    dgb = dgb.astype(BF16)

    w2r = np.zeros((128, NCHUNK * D), np.float32)
    bconv = np.zeros((128, NCHUNK), np.float32)
    for row in range(2048):
        info = rows[row]
        phys = _sem_to_phys(row)
        q, p = divmod(phys, 128)
        if info is not None:
            w2r[p, q * D:(q + 1) * D] = w2[info['cdw']]
            cdw = info['cdw']
            bconv[p, q] = b3[cdw] if cdw < HD // 2 else b5[cdw - HD // 2]
    w2r = w2r.astype(BF16)
    b2rep = np.tile(b2[None, :], (128, 1)).astype(np.float32)

    return dict(w1f=w1f, invn=invn_sb, b1n=b1n_sb, dga=dga, dgb=dgb,
                w2r=w2r, bconv=bconv, b2rep=b2rep)


def _unfold_plan():
    plan = []
    for half in (0, 1):
        base = half * GROWS
        cnt = 0
        for a in range(3):
            for b in range(3):
                for s in range(S[a]):
                    plan.append((half, a, b, s, base + cnt, 20 * S[b]))
                    cnt += 20 * S[b]
    return plan


_UNFOLD_PLAN = _unfold_plan()

_BUILT = None


def _build():
    global _BUILT
    if _BUILT is not None:
        return _BUILT
    import concourse.bacc as bacc
    import concourse.tile as tile
    import concourse.mybir as mybir
    import bass_rust
    from contextlib import ExitStack

    dt = mybir.dt
    AF = mybir.ActivationFunctionType
    OP = mybir.AluOpType

    def view(ap2d, p0, pcnt, off, dims, pstep=1):
        """Arbitrary free-dim view of a [128, F] tile, partitions
        p0, p0+pstep, ... (pcnt of them)."""
        tp = ap2d[p0:p0 + 1, :]
        pitch = ap2d.ap[0][0]
        return bass_rust.AP(tp.tensor, tp.offset + off,
                            [[pitch * pstep, pcnt]]
                            + [[s, c] for s, c in dims])

    nc = bacc.Bacc("TRN2", target_bir_lowering=False, debug=False,
                   enable_asserts=False, num_devices=NCORE)

    x_d = nc.dram_tensor("x_in", [CPC * L, D], dt.float32,
                         kind="ExternalInput").ap()
    w1f_d = nc.dram_tensor("w1f", [128, 21 * 4 * 128], dt.bfloat16,
                           kind="ExternalInput").ap()
    invn_d = nc.dram_tensor("invn", [128, 3 * PMN], dt.bfloat16,
                            kind="ExternalInput").ap()
    b1n_d = nc.dram_tensor("b1n", [128, 3 * PMN], dt.bfloat16,
                           kind="ExternalInput").ap()
    dga_d = nc.dram_tensor("dga", [128, 8 * 9 * 32], dt.bfloat16,
                           kind="ExternalInput").ap()
    dgb_d = nc.dram_tensor("dgb", [128, 8 * 25 * 32], dt.bfloat16,
                           kind="ExternalInput").ap()
    w2r_d = nc.dram_tensor("w2r", [128, NCHUNK * D], dt.bfloat16,
                           kind="ExternalInput").ap()
    bconv_d = nc.dram_tensor("bconv", [128, NCHUNK], dt.float32,
                             kind="ExternalInput").ap()
    b2rep_d = nc.dram_tensor("b2rep", [128, D], dt.float32,
                             kind="ExternalInput").ap()
    out_d = nc.dram_tensor("y_out", [CPC * L, D], dt.float32,
                           kind="ExternalOutput").ap()

    with tile.TileContext(nc) as tc, ExitStack() as ctx:
        dram_pool = ctx.enter_context(
            tc.tile_pool(name="dram", bufs=1, space="DRAM"))
        xbf_d = [dram_pool.tile([L, D], dt.bfloat16, tag=f"xbf{c}",
                                name=f"xbf{c}") for c in range(CPC)]

        consts = ctx.enter_context(tc.tile_pool(name="consts", bufs=1))

        def cload(nm, dram_ap, shape, dtype):
            # consts go on the ACT HWDGE queue so the SP queue is free
            # for the input transposes at startup.
            t = consts.tile(shape, dtype, tag=nm, name=f"c_{nm}")
            nc.scalar.dma_start(t[:, :], dram_ap[:, :])
            return t

        w1f = cload('w1f', w1f_d, [128, 21 * 4 * 128], dt.bfloat16)
        invn = cload('invn', invn_d, [128, 3 * PMN], dt.bfloat16)
        b1n = cload('b1n', b1n_d, [128, 3 * PMN], dt.bfloat16)
        dga = cload('dga', dga_d, [128, 8 * 9 * 32], dt.bfloat16)
        dgb = cload('dgb', dgb_d, [128, 8 * 25 * 32], dt.bfloat16)
        w2r = cload('w2r', w2r_d, [128, NCHUNK * D], dt.bfloat16)
        bconv = cload('bconv', bconv_d, [128, NCHUNK], dt.float32)
        b2rep = cload('b2rep', b2rep_d, [128, D], dt.float32)

        xt_pool = ctx.enter_context(tc.tile_pool(name="xt", bufs=2))
        imgn_pool = ctx.enter_context(tc.tile_pool(name="imgn", bufs=2))
        x2_pool = ctx.enter_context(tc.tile_pool(name="x2", bufs=2))
        g_pool = ctx.enter_context(tc.tile_pool(name="g", bufs=1))
        osb_pool = ctx.enter_context(tc.tile_pool(name="osb", bufs=1))

        # ---------- input path: cast + transpose for both clips ----------
        # fp32->bf16 cast staged through DRAM, split per (clip, kc) column
        # slice so each transpose (and the kc-outer tconv matmuls) can
        # start as soon as its own slice is cast.
        xts = []
        for clip in range(CPC):
            xb = xbf_d[clip][:, :]
            for kc in range(4):
                src = bass_rust.AP(x_d.tensor,
                                   x_d.offset + clip * L * D + kc * 128,
                                   [[D, L], [1, 128]])
                dst = bass_rust.AP(xb.tensor, xb.offset + kc * 128,
                                   [[D, L], [1, 128]])
                nc.gpsimd.dma_start(dst, src)
        for clip in range(CPC):
            xt = xt_pool.tile([128, 4 * L], dt.bfloat16, tag="xt")
            for kc in range(4):
                nc.sync.dma_start(
                    xt[:, kc * L:(kc + 1) * L],
                    xbf_d[clip][:, kc * 128:(kc + 1) * 128],
                    transpose=True)
            xts.append(xt)

        imgns = []
        x2s = []

        def tconv(clip):
            xt = xts[clip]
            imgn = imgn_pool.tile([128, 3 * PMN], dt.bfloat16, tag="imgn")
            # zero the pitch-pad cols (38,39) of every stored row: the
            # contiguous unfold runs sweep them into x2's pad columns.
            nc.vector.memset(view(imgn, 0, 128, PN, [(PNP, 3 * PM), (1, 2)]),
                             0)
            for jh in range(2):
                ps = [tcps.tile([128, 512], dt.float32, tag=f"tc{a}",
                                name=f"tcps{a}")
                      for a in range(3)]
                nmm = {a: sum(1 for (_, _, aa) in TCONV_MMS if aa == a)
                       for a in range(3)}
                # kc-outer: the first 21 matmuls only need xt's kc=0
                # slice, so PE starts as soon as the first transpose
                # lands instead of waiting for the whole xt.
                for k in range(4):
                    cnt = {0: 0, 1: 0, 2: 0}
                    for mi, (s, t, a) in enumerate(TCONV_MMS):
                        cnt[a] += 1
                        if jh == 0:
                            wjd, loc_off, src_off = 19 - t, s * 19 + t, 0
                        else:
                            wjd, loc_off, src_off = 17 + t, s * 19, 19 - t
                        out_ap = view(ps[a], 0, 128, loc_off,
                                      [(19, HOUT), (1, wjd)])
                        rhs = view(xt, 0, 128, k * L + src_off,
                                   [(WOUT, HOUT), (1, wjd)])
                        lhsT = w1f[:, (mi * 4 + k) * 128:
                                   (mi * 4 + k) * 128 + 128]
                        nc.tensor.matmul(out_ap, lhsT, rhs,
                                         start=(cnt[a] == 1 and k == 0),
                                         stop=(cnt[a] == nmm[a]
                                               and k == 3))
                for a in range(3):
                    ps_ap = view(ps[a], 0, 120, 0, [(19, PM), (1, 19)])
                    im_ap = view(imgn, 0, 120, a * PMN + jh * 19,
                                 [(PNP, PM), (1, 19)])
                    iv_ap = view(invn, 0, 120, a * PMN + jh * 19,
                                 [(PNP, PM), (1, 19)])
                    b1_ap = view(b1n, 0, 120, a * PMN + jh * 19,
                                 [(PNP, PM), (1, 19)])
                    nc.vector.tensor_tensor(im_ap, ps_ap, iv_ap, OP.mult)
                    nc.vector.tensor_tensor(im_ap, im_ap, b1_ap, OP.add)
            imgns.append(imgn)

        dma_rr = [0]

        def dma_issue(dst, src, engines=None):
            # alternate the issuing engine: the DMA_DIRECT2D inst occupies
            # its sequencer ~0.6-1.0us regardless of size, so splitting
            # across engines multiplies issue throughput.
            engines = engines or (nc.sync, nc.scalar)
            eng = engines[dma_rr[0] % len(engines)]
            dma_rr[0] += 1
            eng.dma_start(dst, src)

        RUN = (HOUT - 1) * PLW + WOUT  # 796: one contiguous (i,j) sweep

        def unfold(clip):
            # clip 0's DMAs spread over SP+ACT+GpSimd (all idle then, and
            # dw(0) can't start until they land); clip 1's avoid ACT so
            # dw(0)'s gelu ACTIVATEs aren't stuck behind them in the ACT
            # FIFO -- they go to SP+GpSimd instead.
            engines = (nc.sync, nc.scalar, nc.gpsimd) if clip == 0 \
                else (nc.sync, nc.gpsimd)
            imgn = imgns[clip]
            x2a = x2_pool.tile([128, 8 * PLANE], dt.bfloat16, tag="x2a")
            x2b = x2_pool.tile([128, 8 * PLANE], dt.bfloat16, tag="x2b")
            for x2t in (x2a, x2b):
                nc.vector.memset(
                    view(x2t, 0, 128, 0, [(PLANE, 8), (1, 2 * PLW + 2)]), 0)
                nc.vector.memset(
                    view(x2t, 0, 128, 22 * PLW - 2,
                         [(PLANE, 8), (1, 2 * PLW + 2)]), 0)
                # pad rows 980..1023 (chunk 7, partitions 84..127): zero the
                # whole plane so zero-diag matmuls see 0, not NaN garbage.
                # (32-aligned partition base required; unfold rewrites the
                # real rows 960..979 afterwards.)
                nc.vector.memset(x2t[64:128, 7 * PLANE:8 * PLANE], 0)

            # One fan-out DMA per (half, a, b, s): src [nch parts,
            # (t: stride 1, sb), (contig RUN)] -> dst [nch*sb parts,
            # (contig RUN)] (rows ordered (ch, t)), split only at
            # 128-row chunk boundaries. The contiguous runs sweep
            # imgn's zeroed pitch-pad cols into x2's pad columns;
            # the column memset below re-zeroes them afterwards.
            for (half, a, b, s, r0g, n) in _UNFOLD_PLAN:
                sb = S[b]
                x2t = x2a if half == 0 else x2b
                pbase = 40 * b + 20 * half
                r0 = r0g % GROWS
                f = 0
                while f < n:
                    row = r0 + f
                    q, p0 = divmod(row, 128)
                    m = min(n - f, 128 - p0)
                    ch0, t0 = divmod(f, sb)
                    off = a * PMN + s * PNP
                    if t0 != 0 or m < sb:
                        m = min(m, sb - t0)
                        src = view(imgn, pbase + ch0, 1, off + t0,
                                   [(1, m), (1, RUN)])
                    else:
                        m = (m // sb) * sb
                        src = view(imgn, pbase + ch0, m // sb, off,
                                   [(1, sb), (1, RUN)])
                    dst = view(x2t, p0, m, q * PLANE + 2 * PLW + 2,
                               [(1, RUN)])
                    dma_issue(dst, src, engines)
                    f += m
            for x2t in (x2a, x2b):
                nc.vector.memset(
                    view(x2t, 0, 128, 2 * PLW + 38,
                         [(PLANE, 8), (PLW, 20), (1, 4)]), 0)
            x2s.append((x2a, x2b))

        with tc.tile_pool(name="tcps", bufs=2, space="PSUM") as tcps:
            for clip in range(CPC):
                tconv(clip)
                unfold(clip)

        for clip in range(CPC):
            x2a, x2b = x2s[clip]
            # ---------- dwconv ----------
            g = g_pool.tile([128, NCHUNK * L + GPAD], dt.bfloat16, tag="g")
            nc.vector.memset(g[:, NCHUNK * L:NCHUNK * L + GPAD], 0)
            with tc.tile_pool(name="dwps", bufs=2, space="PSUM") as dps:
                for grp in range(2):
                    x2t = x2a if grp == 0 else x2b
                    dg = dga if grp == 0 else dgb
                    nuv = 9 if grp == 0 else 25
                    ko = 1 if grp == 0 else 2
                    uvs = [(du, dv) for du in range(-ko, ko + 1)
                           for dv in range(-ko, ko + 1)]
                    for pg in range(2):
                        for jh in range(2):
                            ps = dps.tile([128, 4 * 512], dt.float32,
                                          tag="dw")
                            for uvi, (du, dv) in enumerate(uvs):
                                for ql in range(4):
                                    q = 4 * pg + ql
                                    for r in range(4):
                                        c = (r + q + 8 * grp) % 4
                                        lhsT = dg[32 * r:32 * r + 32,
                                                  (q * nuv + uvi) * 32:
                                                  (q * nuv + uvi + 1) * 32]
                                        rhs = view(
                                            x2t, 32 * r, 32,
                                            q * PLANE + (2 + du) * PLW
                                            + 2 + dv + jh * 18,
                                            [(PLW, HOUT), (1, 18)])
                                        out = view(ps, 32 * c, 32, ql * 512,
                                                   [(18, HOUT), (1, 18)])
                                        nc.tensor.matmul(
                                            out, lhsT, rhs,
                                            start=(uvi == 0),
                                            stop=(uvi == nuv - 1),
                                            tile_position=(32 * r, 32 * c))
                            for ql in range(4):
                                gq = 8 * grp + 4 * pg + ql
                                g_ap = view(g, 0, 128, gq * L + jh * 18,
                                            [(WOUT, HOUT), (1, 18)])
                                ps_ap = view(ps, 0, 128, ql * 512,
                                             [(18, HOUT), (1, 18)])
                                nc.scalar.activation(
                                    g_ap, ps_ap, AF.Gelu,
                                    bias=bconv[:, gq:gq + 1], scale=1.0)

            # ---------- mm2 ----------
            with tc.tile_pool(name="mmps", bufs=2, space="PSUM") as mps:
                osb = osb_pool.tile([128, 6 * D], dt.float32, tag="osb")
                for mt in range(6):
                    pso = mps.tile([128, D], dt.float32, tag="mm2")
                    for kc in range(NCHUNK):
                        lhsT = g[:, kc * L + mt * 120:kc * L + mt * 120 + 128]
                        rhs = w2r[:, kc * D:(kc + 1) * D]
                        nc.tensor.matmul(pso[:, :], lhsT, rhs,
                                         start=(kc == 0),
                                         stop=(kc == NCHUNK - 1))
                    nc.vector.tensor_tensor(osb[0:120, mt * D:(mt + 1) * D],
                                            pso[0:120, :],
                                            b2rep[0:120, :], OP.add)
                    if mt in (2, 5):
                        # two DMAs per clip: first half ships while the
                        # second half's matmuls still run. src order
                        # (p, mt, d) matches dst rows mt*120+p via 3-dim AP.
                        mt0 = mt - 2
                        src = view(osb, 0, 120, mt0 * D, [(D, 3), (1, D)])
                        dst = bass_rust.AP(
                            out_d.tensor,
                            out_d.offset + (clip * L + mt0 * 120) * D,
                            [[D, 120], [120 * D, 3], [1, D]])
                        nc.sync.dma_start(dst, src)

    nc.compile()
    _BUILT = nc
    return nc


def kernel(**inputs):
    x = np.asarray(inputs['x'], np.float32)
    consts = build_consts(
        np.asarray(inputs['w1'], np.float32),
        np.asarray(inputs['b1'], np.float32),
        np.asarray(inputs['w3'], np.float32),
        np.asarray(inputs['b3'], np.float32),
        np.asarray(inputs['w5'], np.float32),
        np.asarray(inputs['b5'], np.float32),
        np.asarray(inputs['w2'], np.float32),
        np.asarray(inputs['b2'], np.float32))
    nc = _build()
    from concourse.bass_utils import run_bass_kernel_spmd

    xf = x.reshape(NCLIP, L, D)
    in_maps = []
    for core in range(NCORE):
        m = {k: consts[k] for k in consts}
        m['x_in'] = np.ascontiguousarray(
            xf[core * CPC:(core + 1) * CPC].reshape(CPC * L, D))
        in_maps.append(m)
    res = run_bass_kernel_spmd(nc, in_maps, core_ids=list(range(NCORE)))
    out = np.zeros((NCLIP, L, D), np.float32)
    for core in range(NCORE):
        out[core * CPC:(core + 1) * CPC] = \
            res.results[core]['y_out'].reshape(CPC, L, D)
    return out.reshape(B, T * L, D)


# revision 3
# speedup vs baseline: 1.0364x; 1.0015x over previous
"""Trainium2 Bass kernel for nn_MixFusionFeedForward (self-contained).

Data-parallel over the 16 video clips (2 per NeuronCore). Per clip:
  x[720,512] --DMA cast+transpose--> xT bf16 [512,720]
  "tconv": phase-decomposed stride-3 transposed conv == matmul1 + fold fused:
     img[(a,b)][ch, m, n] = sum_{s<S_a, t<S_b} (x @ w1[:,ch,a+3s,b+3t])[m-s,n-t]
     as 21 shifted matmuls accumulating in PSUM (bf16 in, fp32 accum).
  norm: imgn = img * invnorm + b1*invnorm  (DVE, PSUM -> SBUF bf16)
  unfold: X2[(phase,s,ch,t)][i,j] = imgn[phase][ch, i+s, j+t]  (SBUF->SBUF DMA)
  dwconv: depthwise 3x3/5x5 = diagonal 32x32 matmuls, 16 PE sub-array tiles
     concurrent (tile_position packing), taps accumulate in PSUM.
  gelu(+conv bias) on ScalarE evacuating PSUM -> g bf16
  mm2: out[l,:] = g.T @ w2r + b2 (bf16 matmul, fp32 accum)

v2 scheduling: both clips' input DMAs issued up front; tconv(0), tconv(1)
run back-to-back on PE while unfold(clip) DMAs (issued right after each
tconv) overlap the other clip's matmuls; then dw(0), mm2(0), dw(1), mm2(1).
Weight tiles padded to 128 columns so the compiler's fast-weight-load path
(FWL, needs NumWeights==128 and non-fp32) engages for tconv and mm2.
"""
import sys
if '/opt/trn_rl_repo' not in sys.path:
    sys.path.insert(0, '/opt/trn_rl_repo')

import numpy as np
import ml_dtypes

D = 512
HD = 1960
NCH = 40
KH = KW = 7
HOUT, WOUT = 20, 36
L = HOUT * WOUT
T = 8
B = 2
NCLIP = B * T
NCORE = 8
CPC = NCLIP // NCORE
PM, PN = 22, 38
PNP = 40                  # stored imgn row pitch (cols 38,39 zeroed)
PMN = PM * PNP
S = (3, 2, 2)
IH, IW = 60, 108
PH = PW = 3
PLH, PLW = 24, 40
PLANE = PLH * PLW
GROWS = 1024
NCHUNK = 16
GPAD = 8
BF16 = ml_dtypes.bfloat16

TCONV_MMS = [(s, t, a) for s in range(3) for t in range(3) for a in range(3)
             if s < S[a]]
assert len(TCONV_MMS) == 21


def _sem_rows():
    rows = []
    for half in (0, 1):
        cnt = 0
        for a in range(3):
            for b in range(3):
                for s in range(S[a]):
                    for ch in range(20):
                        for t in range(S[b]):
                            ki, kj = a + 3 * s, b + 3 * t
                            chfull = half * 20 + ch
                            rows.append(dict(half=half, a=a, b=b, s=s, ch=ch,
                                             t=t, cdw=chfull * 49 + ki * 7 + kj))
                            cnt += 1
        assert cnt == 980
        rows.extend([None] * (GROWS - 980))
    return rows


def _sem_to_phys(row):
    q, p = divmod(row, 128)
    r, o = divmod(p, 32)
    return q * 128 + 32 * ((r + q) % 4) + o


def build_consts(w1, b1, w3, b3, w5, b5, w2, b2):
    rows = _sem_rows()
    w1r = w1.reshape(D, NCH, KH, KW)
    b1r = b1.reshape(NCH, KH, KW)

    # tconv weights, padded to 128 columns per slice for FWL. Laid out
    # (k, mi) so each k-pass of the kc-outer matmul loop reads one
    # contiguous quarter -> the 4-piece w1f load unblocks PE early.
    w1f = np.zeros((128, len(TCONV_MMS) * 4 * 128), np.float32)
    for mi, (s, t, a) in enumerate(TCONV_MMS):
        tl = np.zeros((D, 128), np.float32)
        for b_ in range(3):
            if t >= S[b_]:
                continue
            for half in (0, 1):
                cols = 40 * b_ + 20 * half + np.arange(20)
                tl[:, cols] = w1r[:, half * 20:half * 20 + 20,
                                  a + 3 * s, b_ + 3 * t]
        for k in range(4):
            w1f[:, (k * 21 + mi) * 128:(k * 21 + mi) * 128 + 128] = \
                tl[k * 128:(k + 1) * 128]
    w1f = w1f.astype(BF16)

    nr = np.zeros(3 * PM)
    ncv = np.zeros(3 * PN)
    for i in range(HOUT):
        nr[3 * i:3 * i + KH] += 1
    for j in range(WOUT):
        ncv[3 * j:3 * j + KW] += 1
    invn = np.zeros((3, 3, PM, PN), np.float32)
    for a in range(3):
        for b_ in range(3):
            r = 3 * np.arange(PM) + a
            c = 3 * np.arange(PN) + b_
            rv = (r >= PH) & (r <= IH + PH - 1)
            cv = (c >= PW) & (c <= IW + PW - 1)
            with np.errstate(divide='ignore'):
                iv = 1.0 / np.outer(nr[r], ncv[c])
            iv[~rv, :] = 0
            iv[:, ~cv] = 0
            invn[a, b_] = iv
    b1img = np.zeros((NCH, 3, 3, PM, PN), np.float32)
    for a in range(3):
        for b_ in range(3):
            for s in range(S[a]):
                for t in range(S[b_]):
                    v = b1r[:, a + 3 * s, b_ + 3 * t]
                    b1img[:, a, b_, s:s + HOUT, t:t + WOUT] += v[:, None, None]
    invn_sb = np.zeros((128, 3 * PMN), np.float32)
    b1n_sb = np.zeros((128, 3 * PMN), np.float32)
    for a in range(3):
        for b_ in range(3):
            for half in (0, 1):
                for ch in range(20):
                    p = 40 * b_ + 20 * half + ch
                    iv = np.zeros((PM, PNP), np.float32)
                    iv[:, :PN] = invn[a, b_]
                    bv = np.zeros((PM, PNP), np.float32)
                    bv[:, :PN] = b1img[half * 20 + ch, a, b_] * invn[a, b_]
                    invn_sb[p, a * PMN:(a + 1) * PMN] = iv.ravel()
                    b1n_sb[p, a * PMN:(a + 1) * PMN] = bv.ravel()
    invn_sb = invn_sb.astype(BF16)
    b1n_sb = b1n_sb.astype(BF16)

    def dw_weight(row, du, dv, k, off):
        info = rows[row]
        if info is None:
            return 0.0
        w = w3[info['cdw'], 0] if k == 3 else w5[info['cdw'] - HD // 2, 0]
        return float(w[du + off, dv + off])

    dga = np.zeros((128, 8 * 9 * 32), np.float32)
    dgb = np.zeros((128, 8 * 25 * 32), np.float32)
    for q in range(8):
        for r in range(4):
            for o in range(32):
                rowa = q * 128 + 32 * r + o
                rowb = (8 + q) * 128 + 32 * r + o
                for uvi, (du, dv) in enumerate(
                        (du, dv) for du in (-1, 0, 1) for dv in (-1, 0, 1)):
                    dga[32 * r + o, (q * 9 + uvi) * 32 + o] = \
                        dw_weight(rowa, du, dv, 3, 1)
                for uvi, (du, dv) in enumerate(
                        (du, dv) for du in (-2, -1, 0, 1, 2)
                        for dv in (-2, -1, 0, 1, 2)):
                    dgb[32 * r + o, (q * 25 + uvi) * 32 + o] = \
                        dw_weight(rowb, du, dv, 5, 2)
    dga = dga.astype(BF16)
    dgb = dgb.astype(BF16)

    w2r = np.zeros((128, NCHUNK * D), np.float32)
    bconv = np.zeros((128, NCHUNK), np.float32)
    for row in range(2048):
        info = rows[row]
        phys = _sem_to_phys(row)
        q, p = divmod(phys, 128)
        if info is not None:
            w2r[p, q * D:(q + 1) * D] = w2[info['cdw']]
            cdw = info['cdw']
            bconv[p, q] = b3[cdw] if cdw < HD // 2 else b5[cdw - HD // 2]
    w2r = w2r.astype(BF16)
    b2rep = np.tile(b2[None, :], (128, 1)).astype(np.float32)

    return dict(w1f=w1f, invn=invn_sb, b1n=b1n_sb, dga=dga, dgb=dgb,
                w2r=w2r, bconv=bconv, b2rep=b2rep)


def _unfold_plan():
    plan = []
    for half in (0, 1):
        base = half * GROWS
        cnt = 0
        for a in range(3):
            for b in range(3):
                for s in range(S[a]):
                    plan.append((half, a, b, s, base + cnt, 20 * S[b]))
                    cnt += 20 * S[b]
    return plan


_UNFOLD_PLAN = _unfold_plan()

_BUILT = None


def _build():
    global _BUILT
    if _BUILT is not None:
        return _BUILT
    import concourse.bacc as bacc
    import concourse.tile as tile
    import concourse.mybir as mybir
    import bass_rust
    from contextlib import ExitStack

    dt = mybir.dt
    AF = mybir.ActivationFunctionType
    OP = mybir.AluOpType

    def view(ap2d, p0, pcnt, off, dims, pstep=1):
        """Arbitrary free-dim view of a [128, F] tile, partitions
        p0, p0+pstep, ... (pcnt of them)."""
        tp = ap2d[p0:p0 + 1, :]
        pitch = ap2d.ap[0][0]
        return bass_rust.AP(tp.tensor, tp.offset + off,
                            [[pitch * pstep, pcnt]]
                            + [[s, c] for s, c in dims])

    nc = bacc.Bacc("TRN2", target_bir_lowering=False, debug=False,
                   enable_asserts=False, num_devices=NCORE)

    x_d = nc.dram_tensor("x_in", [CPC * L, D], dt.float32,
                         kind="ExternalInput").ap()
    w1f_d = nc.dram_tensor("w1f", [128, 21 * 4 * 128], dt.bfloat16,
                           kind="ExternalInput").ap()
    invn_d = nc.dram_tensor("invn", [128, 3 * PMN], dt.bfloat16,
                            kind="ExternalInput").ap()
    b1n_d = nc.dram_tensor("b1n", [128, 3 * PMN], dt.bfloat16,
                           kind="ExternalInput").ap()
    dga_d = nc.dram_tensor("dga", [128, 8 * 9 * 32], dt.bfloat16,
                           kind="ExternalInput").ap()
    dgb_d = nc.dram_tensor("dgb", [128, 8 * 25 * 32], dt.bfloat16,
                           kind="ExternalInput").ap()
    w2r_d = nc.dram_tensor("w2r", [128, NCHUNK * D], dt.bfloat16,
                           kind="ExternalInput").ap()
    bconv_d = nc.dram_tensor("bconv", [128, NCHUNK], dt.float32,
                             kind="ExternalInput").ap()
    b2rep_d = nc.dram_tensor("b2rep", [128, D], dt.float32,
                             kind="ExternalInput").ap()
    out_d = nc.dram_tensor("y_out", [CPC * L, D], dt.float32,
                           kind="ExternalOutput").ap()

    with tile.TileContext(nc) as tc, ExitStack() as ctx:
        dram_pool = ctx.enter_context(
            tc.tile_pool(name="dram", bufs=1, space="DRAM"))
        xbf_d = [dram_pool.tile([L, D], dt.bfloat16, tag=f"xbf{c}",
                                name=f"xbf{c}") for c in range(CPC)]

        consts = ctx.enter_context(tc.tile_pool(name="consts", bufs=1))

        def cload(nm, dram_ap, shape, dtype):
            # consts go on the ACT HWDGE queue so the SP queue is free
            # for the input transposes at startup.
            t = consts.tile(shape, dtype, tag=nm, name=f"c_{nm}")
            nc.scalar.dma_start(t[:, :], dram_ap[:, :])
            return t

        w1f = consts.tile([128, 21 * 4 * 128], dt.bfloat16, tag='w1f',
                          name='c_w1f')
        for k in range(4):
            kw = 21 * 128
            nc.scalar.dma_start(w1f[:, k * kw:(k + 1) * kw],
                                w1f_d[:, k * kw:(k + 1) * kw])
        invn = cload('invn', invn_d, [128, 3 * PMN], dt.bfloat16)
        b1n = cload('b1n', b1n_d, [128, 3 * PMN], dt.bfloat16)
        dga = cload('dga', dga_d, [128, 8 * 9 * 32], dt.bfloat16)
        dgb = cload('dgb', dgb_d, [128, 8 * 25 * 32], dt.bfloat16)
        w2r = cload('w2r', w2r_d, [128, NCHUNK * D], dt.bfloat16)
        bconv = cload('bconv', bconv_d, [128, NCHUNK], dt.float32)
        b2rep = cload('b2rep', b2rep_d, [128, D], dt.float32)

        xt_pool = ctx.enter_context(tc.tile_pool(name="xt", bufs=2))
        imgn_pool = ctx.enter_context(tc.tile_pool(name="imgn", bufs=2))
        x2_pool = ctx.enter_context(tc.tile_pool(name="x2", bufs=2))
        g_pool = ctx.enter_context(tc.tile_pool(name="g", bufs=1))
        osb_pool = ctx.enter_context(tc.tile_pool(name="osb", bufs=1))

        # ---------- input path: cast + transpose for both clips ----------
        # fp32->bf16 cast staged through DRAM, split per (clip, kc) column
        # slice so each transpose (and the kc-outer tconv matmuls) can
        # start as soon as its own slice is cast.
        xts = []
        for clip in range(CPC):
            # contiguous row-halves: few big descriptors, so the SWDGE
            # emission doesn't sit on the critical path like column
            # slices (512B runs x 720) did.
            xb = xbf_d[clip][:, :]
            for rh in range(2):
                o = rh * (L // 2) * D
                src = bass_rust.AP(x_d.tensor,
                                   x_d.offset + clip * L * D + o,
                                   [[D, L // 2], [1, D]])
                dst = bass_rust.AP(xb.tensor, xb.offset + o,
                                   [[D, L // 2], [1, D]])
                nc.gpsimd.dma_start(dst, src)
        for clip in range(CPC):
            xt = xt_pool.tile([128, 4 * L], dt.bfloat16, tag="xt")
            for kc in range(4):
                nc.sync.dma_start(
                    xt[:, kc * L:(kc + 1) * L],
                    xbf_d[clip][:, kc * 128:(kc + 1) * 128],
                    transpose=True)
            xts.append(xt)

        imgns = []
        x2s = []

        def tconv(clip):
            xt = xts[clip]
            imgn = imgn_pool.tile([128, 3 * PMN], dt.bfloat16, tag="imgn")
            # zero the pitch-pad cols (38,39) of every stored row: the
            # contiguous unfold runs sweep them into x2's pad columns.
            nc.vector.memset(view(imgn, 0, 128, PN, [(PNP, 3 * PM), (1, 2)]),
                             0)
            for jh in range(2):
                ps = [tcps.tile([128, 512], dt.float32, tag=f"tc{a}",
                                name=f"tcps{a}")
                      for a in range(3)]
                nmm = {a: sum(1 for (_, _, aa) in TCONV_MMS if aa == a)
                       for a in range(3)}
                # kc-outer: the first 21 matmuls only need xt's kc=0
                # slice, so PE starts as soon as the first transpose
                # lands instead of waiting for the whole xt.
                for k in range(4):
                    cnt = {0: 0, 1: 0, 2: 0}
                    for mi, (s, t, a) in enumerate(TCONV_MMS):
                        cnt[a] += 1
                        if jh == 0:
                            wjd, loc_off, src_off = 19 - t, s * 19 + t, 0
                        else:
                            wjd, loc_off, src_off = 17 + t, s * 19, 19 - t
                        out_ap = view(ps[a], 0, 128, loc_off,
                                      [(19, HOUT), (1, wjd)])
                        rhs = view(xt, 0, 128, k * L + src_off,
                                   [(WOUT, HOUT), (1, wjd)])
                        lhsT = w1f[:, (k * 21 + mi) * 128:
                                   (k * 21 + mi) * 128 + 128]
                        nc.tensor.matmul(out_ap, lhsT, rhs,
                                         start=(cnt[a] == 1 and k == 0),
                                         stop=(cnt[a] == nmm[a]
                                               and k == 3))
                for a in range(3):
                    ps_ap = view(ps[a], 0, 120, 0, [(19, PM), (1, 19)])
                    im_ap = view(imgn, 0, 120, a * PMN + jh * 19,
                                 [(PNP, PM), (1, 19)])
                    iv_ap = view(invn, 0, 120, a * PMN + jh * 19,
                                 [(PNP, PM), (1, 19)])
                    b1_ap = view(b1n, 0, 120, a * PMN + jh * 19,
                                 [(PNP, PM), (1, 19)])
                    nc.vector.tensor_tensor(im_ap, ps_ap, iv_ap, OP.mult)
                    nc.vector.tensor_tensor(im_ap, im_ap, b1_ap, OP.add)
            imgns.append(imgn)

        dma_rr = [0]

        def dma_issue(dst, src, engines=None):
            # alternate the issuing engine: the DMA_DIRECT2D inst occupies
            # its sequencer ~0.6-1.0us regardless of size, so splitting
            # across engines multiplies issue throughput.
            engines = engines or (nc.sync, nc.scalar)
            eng = engines[dma_rr[0] % len(engines)]
            dma_rr[0] += 1
            eng.dma_start(dst, src)

        RUN = (HOUT - 1) * PLW + WOUT  # 796: one contiguous (i,j) sweep

        def unfold(clip):
            # clip 0's DMAs spread over SP+ACT+GpSimd (all idle then, and
            # dw(0) can't start until they land); clip 1's avoid ACT so
            # dw(0)'s gelu ACTIVATEs aren't stuck behind them in the ACT
            # FIFO -- they go to SP+GpSimd instead.
            engines = (nc.sync, nc.scalar, nc.gpsimd) if clip == 0 \
                else (nc.sync, nc.gpsimd)
            imgn = imgns[clip]
            x2a = x2_pool.tile([128, 8 * PLANE], dt.bfloat16, tag="x2a")
            x2b = x2_pool.tile([128, 8 * PLANE], dt.bfloat16, tag="x2b")
            for x2t in (x2a, x2b):
                nc.vector.memset(
                    view(x2t, 0, 128, 0, [(PLANE, 8), (1, 2 * PLW + 2)]), 0)
                nc.vector.memset(
                    view(x2t, 0, 128, 22 * PLW - 2,
                         [(PLANE, 8), (1, 2 * PLW + 2)]), 0)
                # pad rows 980..1023 (chunk 7, partitions 84..127): zero the
                # whole plane so zero-diag matmuls see 0, not NaN garbage.
                # (32-aligned partition base required; unfold rewrites the
                # real rows 960..979 afterwards.)
                nc.vector.memset(x2t[64:128, 7 * PLANE:8 * PLANE], 0)

            # One fan-out DMA per (half, a, b, s): src [nch parts,
            # (t: stride 1, sb), (contig RUN)] -> dst [nch*sb parts,
            # (contig RUN)] (rows ordered (ch, t)), split only at
            # 128-row chunk boundaries. The contiguous runs sweep
            # imgn's zeroed pitch-pad cols into x2's pad columns;
            # the column memset below re-zeroes them afterwards.
            for (half, a, b, s, r0g, n) in _UNFOLD_PLAN:
                sb = S[b]
                x2t = x2a if half == 0 else x2b
                pbase = 40 * b + 20 * half
                r0 = r0g % GROWS
                f = 0
                while f < n:
                    row = r0 + f
                    q, p0 = divmod(row, 128)
                    m = min(n - f, 128 - p0)
                    ch0, t0 = divmod(f, sb)
                    off = a * PMN + s * PNP
                    if t0 != 0 or m < sb:
                        m = min(m, sb - t0)
                        src = view(imgn, pbase + ch0, 1, off + t0,
                                   [(1, m), (1, RUN)])
                    else:
                        m = (m // sb) * sb
                        src = view(imgn, pbase + ch0, m // sb, off,
                                   [(1, sb), (1, RUN)])
                    dst = view(x2t, p0, m, q * PLANE + 2 * PLW + 2,
                               [(1, RUN)])
                    dma_issue(dst, src, engines)
                    f += m
            for x2t in (x2a, x2b):
                nc.vector.memset(
                    view(x2t, 0, 128, 2 * PLW + 38,
                         [(PLANE, 8), (PLW, 20), (1, 4)]), 0)
            x2s.append((x2a, x2b))

        with tc.tile_pool(name="tcps", bufs=2, space="PSUM") as tcps:
            for clip in range(CPC):
                tconv(clip)
                unfold(clip)

        for clip in range(CPC):
            x2a, x2b = x2s[clip]
            # ---------- dwconv ----------
            g = g_pool.tile([128, NCHUNK * L + GPAD], dt.bfloat16, tag="g")
            nc.vector.memset(g[:, NCHUNK * L:NCHUNK * L + GPAD], 0)
            with tc.tile_pool(name="dwps", bufs=2, space="PSUM") as dps:
                for grp in range(2):
                    x2t = x2a if grp == 0 else x2b
                    dg = dga if grp == 0 else dgb
                    nuv = 9 if grp == 0 else 25
                    ko = 1 if grp == 0 else 2
                    uvs = [(du, dv) for du in range(-ko, ko + 1)
                           for dv in range(-ko, ko + 1)]
                    for pg in range(2):
                        for jh in range(2):
                            ps = dps.tile([128, 4 * 512], dt.float32,
                                          tag="dw")
                            for uvi, (du, dv) in enumerate(uvs):
                                for ql in range(4):
                                    q = 4 * pg + ql
                                    for r in range(4):
                                        c = (r + q + 8 * grp) % 4
                                        lhsT = dg[32 * r:32 * r + 32,
                                                  (q * nuv + uvi) * 32:
                                                  (q * nuv + uvi + 1) * 32]
                                        rhs = view(
                                            x2t, 32 * r, 32,
                                            q * PLANE + (2 + du) * PLW
                                            + 2 + dv + jh * 18,
                                            [(PLW, HOUT), (1, 18)])
                                        out = view(ps, 32 * c, 32, ql * 512,
                                                   [(18, HOUT), (1, 18)])
                                        nc.tensor.matmul(
                                            out, lhsT, rhs,
                                            start=(uvi == 0),
                                            stop=(uvi == nuv - 1),
                                            tile_position=(32 * r, 32 * c))
                            for ql in range(4):
                                gq = 8 * grp + 4 * pg + ql
                                g_ap = view(g, 0, 128, gq * L + jh * 18,
                                            [(WOUT, HOUT), (1, 18)])
                                ps_ap = view(ps, 0, 128, ql * 512,
                                             [(18, HOUT), (1, 18)])
                                nc.scalar.activation(
                                    g_ap, ps_ap, AF.Gelu,
                                    bias=bconv[:, gq:gq + 1], scale=1.0)

            # ---------- mm2 ----------
            with tc.tile_pool(name="mmps", bufs=2, space="PSUM") as mps:
                osb = osb_pool.tile([128, 6 * D], dt.float32, tag="osb")
                for mt in range(6):
                    pso = mps.tile([128, D], dt.float32, tag="mm2")
                    for kc in range(NCHUNK):
                        lhsT = g[:, kc * L + mt * 120:kc * L + mt * 120 + 128]
                        rhs = w2r[:, kc * D:(kc + 1) * D]
                        nc.tensor.matmul(pso[:, :], lhsT, rhs,
                                         start=(kc == 0),
                                         stop=(kc == NCHUNK - 1))
                    nc.vector.tensor_tensor(osb[0:120, mt * D:(mt + 1) * D],
                                            pso[0:120, :],
                                            b2rep[0:120, :], OP.add)
                    if mt in (2, 5):
                        # two DMAs per clip: first half ships while the
                        # second half's matmuls still run. src order
                        # (p, mt, d) matches dst rows mt*120+p via 3-dim AP.
                        mt0 = mt - 2
                        src = view(osb, 0, 120, mt0 * D, [(D, 3), (1, D)])
                        dst = bass_rust.AP(
                            out_d.tensor,
                            out_d.offset + (clip * L + mt0 * 120) * D,
                            [[D, 120], [120 * D, 3], [1, D]])
                        nc.sync.dma_start(dst, src)

    nc.compile()
    _BUILT = nc
    return nc


def kernel(**inputs):
    x = np.asarray(inputs['x'], np.float32)
    consts = build_consts(
        np.asarray(inputs['w1'], np.float32),
        np.asarray(inputs['b1'], np.float32),
        np.asarray(inputs['w3'], np.float32),
        np.asarray(inputs['b3'], np.float32),
        np.asarray(inputs['w5'], np.float32),
        np.asarray(inputs['b5'], np.float32),
        np.asarray(inputs['w2'], np.float32),
        np.asarray(inputs['b2'], np.float32))
    nc = _build()
    from concourse.bass_utils import run_bass_kernel_spmd

    xf = x.reshape(NCLIP, L, D)
    in_maps = []
    for core in range(NCORE):
        m = {k: consts[k] for k in consts}
        m['x_in'] = np.ascontiguousarray(
            xf[core * CPC:(core + 1) * CPC].reshape(CPC * L, D))
        in_maps.append(m)
    res = run_bass_kernel_spmd(nc, in_maps, core_ids=list(range(NCORE)))
    out = np.zeros((NCLIP, L, D), np.float32)
    for core in range(NCORE):
        out[core * CPC:(core + 1) * CPC] = \
            res.results[core]['y_out'].reshape(CPC, L, D)
    return out.reshape(B, T * L, D)


# revision 4
# speedup vs baseline: 1.0476x; 1.0107x over previous
"""Trainium2 Bass kernel for nn_MixFusionFeedForward (self-contained).

Data-parallel over the 16 video clips (2 per NeuronCore). Per clip:
  x[720,512] --DMA cast+transpose--> xT bf16 [512,720]
  "tconv": phase-decomposed stride-3 transposed conv == matmul1 + fold fused:
     img[(a,b)][ch, m, n] = sum_{s<S_a, t<S_b} (x @ w1[:,ch,a+3s,b+3t])[m-s,n-t]
     as 21 shifted matmuls accumulating in PSUM (bf16 in, fp32 accum).
  norm: imgn = img * invnorm + b1*invnorm  (DVE, PSUM -> SBUF bf16)
  unfold: X2[(phase,s,ch,t)][i,j] = imgn[phase][ch, i+s, j+t]  (SBUF->SBUF DMA)
  dwconv: depthwise 3x3/5x5 = diagonal 32x32 matmuls, 16 PE sub-array tiles
     concurrent (tile_position packing), taps accumulate in PSUM.
  gelu(+conv bias) on ScalarE evacuating PSUM -> g bf16
  mm2: out[l,:] = g.T @ w2r + b2 (bf16 matmul, fp32 accum)

v2 scheduling: both clips' input DMAs issued up front; tconv(0), tconv(1)
run back-to-back on PE while unfold(clip) DMAs (issued right after each
tconv) overlap the other clip's matmuls; then dw(0), mm2(0), dw(1), mm2(1).
Weight tiles padded to 128 columns so the compiler's fast-weight-load path
(FWL, needs NumWeights==128 and non-fp32) engages for tconv and mm2.
"""
import sys
if '/opt/trn_rl_repo' not in sys.path:
    sys.path.insert(0, '/opt/trn_rl_repo')

import numpy as np
import ml_dtypes

D = 512
HD = 1960
NCH = 40
KH = KW = 7
HOUT, WOUT = 20, 36
L = HOUT * WOUT
T = 8
B = 2
NCLIP = B * T
NCORE = 8
CPC = NCLIP // NCORE
PM, PN = 22, 38
PNP = 40                  # stored imgn row pitch (cols 38,39 zeroed)
PMN = PM * PNP
S = (3, 2, 2)
IH, IW = 60, 108
PH = PW = 3
PLH, PLW = 24, 40
PLANE = PLH * PLW
GROWS = 1024
NCHUNK = 16
GPAD = 8
BF16 = ml_dtypes.bfloat16

TCONV_MMS = [(s, t, a) for s in range(3) for t in range(3) for a in range(3)
             if s < S[a]]
assert len(TCONV_MMS) == 21


def _sem_rows():
    rows = []
    for half in (0, 1):
        cnt = 0
        for a in range(3):
            for b in range(3):
                for s in range(S[a]):
                    for ch in range(20):
                        for t in range(S[b]):
                            ki, kj = a + 3 * s, b + 3 * t
                            chfull = half * 20 + ch
                            rows.append(dict(half=half, a=a, b=b, s=s, ch=ch,
                                             t=t, cdw=chfull * 49 + ki * 7 + kj))
                            cnt += 1
        assert cnt == 980
        rows.extend([None] * (GROWS - 980))
    return rows


def _sem_to_phys(row):
    q, p = divmod(row, 128)
    r, o = divmod(p, 32)
    return q * 128 + 32 * ((r + q) % 4) + o


def build_consts(w1, b1, w3, b3, w5, b5, w2, b2):
    rows = _sem_rows()
    w1r = w1.reshape(D, NCH, KH, KW)
    b1r = b1.reshape(NCH, KH, KW)

    # tconv weights, padded to 128 columns per slice for FWL. Laid out
    # (k, mi) so each k-pass of the kc-outer matmul loop reads one
    # contiguous quarter -> the 4-piece w1f load unblocks PE early.
    w1f = np.zeros((128, len(TCONV_MMS) * 4 * 128), np.float32)
    for mi, (s, t, a) in enumerate(TCONV_MMS):
        tl = np.zeros((D, 128), np.float32)
        for b_ in range(3):
            if t >= S[b_]:
                continue
            for half in (0, 1):
                cols = 40 * b_ + 20 * half + np.arange(20)
                tl[:, cols] = w1r[:, half * 20:half * 20 + 20,
                                  a + 3 * s, b_ + 3 * t]
        for k in range(4):
            w1f[:, (k * 21 + mi) * 128:(k * 21 + mi) * 128 + 128] = \
                tl[k * 128:(k + 1) * 128]
    w1f = w1f.astype(BF16)

    nr = np.zeros(3 * PM)
    ncv = np.zeros(3 * PN)
    for i in range(HOUT):
        nr[3 * i:3 * i + KH] += 1
    for j in range(WOUT):
        ncv[3 * j:3 * j + KW] += 1
    invn = np.zeros((3, 3, PM, PN), np.float32)
    for a in range(3):
        for b_ in range(3):
            r = 3 * np.arange(PM) + a
            c = 3 * np.arange(PN) + b_
            rv = (r >= PH) & (r <= IH + PH - 1)
            cv = (c >= PW) & (c <= IW + PW - 1)
            with np.errstate(divide='ignore'):
                iv = 1.0 / np.outer(nr[r], ncv[c])
            iv[~rv, :] = 0
            iv[:, ~cv] = 0
            invn[a, b_] = iv
    b1img = np.zeros((NCH, 3, 3, PM, PN), np.float32)
    for a in range(3):
        for b_ in range(3):
            for s in range(S[a]):
                for t in range(S[b_]):
                    v = b1r[:, a + 3 * s, b_ + 3 * t]
                    b1img[:, a, b_, s:s + HOUT, t:t + WOUT] += v[:, None, None]
    invn_sb = np.zeros((128, 3 * PMN), np.float32)
    b1n_sb = np.zeros((128, 3 * PMN), np.float32)
    for a in range(3):
        for b_ in range(3):
            for half in (0, 1):
                for ch in range(20):
                    p = 40 * b_ + 20 * half + ch
                    iv = np.zeros((PM, PNP), np.float32)
                    iv[:, :PN] = invn[a, b_]
                    bv = np.zeros((PM, PNP), np.float32)
                    bv[:, :PN] = b1img[half * 20 + ch, a, b_] * invn[a, b_]
                    invn_sb[p, a * PMN:(a + 1) * PMN] = iv.ravel()
                    b1n_sb[p, a * PMN:(a + 1) * PMN] = bv.ravel()
    invn_sb = invn_sb.astype(BF16)
    b1n_sb = b1n_sb.astype(BF16)

    def dw_weight(row, du, dv, k, off):
        info = rows[row]
        if info is None:
            return 0.0
        w = w3[info['cdw'], 0] if k == 3 else w5[info['cdw'] - HD // 2, 0]
        return float(w[du + off, dv + off])

    dga = np.zeros((128, 8 * 9 * 32), np.float32)
    dgb = np.zeros((128, 8 * 25 * 32), np.float32)
    for q in range(8):
        for r in range(4):
            for o in range(32):
                rowa = q * 128 + 32 * r + o
                rowb = (8 + q) * 128 + 32 * r + o
                for uvi, (du, dv) in enumerate(
                        (du, dv) for du in (-1, 0, 1) for dv in (-1, 0, 1)):
                    dga[32 * r + o, (q * 9 + uvi) * 32 + o] = \
                        dw_weight(rowa, du, dv, 3, 1)
                for uvi, (du, dv) in enumerate(
                        (du, dv) for du in (-2, -1, 0, 1, 2)
                        for dv in (-2, -1, 0, 1, 2)):
                    dgb[32 * r + o, (q * 25 + uvi) * 32 + o] = \
                        dw_weight(rowb, du, dv, 5, 2)
    dga = dga.astype(BF16)
    dgb = dgb.astype(BF16)

    w2r = np.zeros((128, NCHUNK * D), np.float32)
    bconv = np.zeros((128, NCHUNK), np.float32)
    for row in range(2048):
        info = rows[row]
        phys = _sem_to_phys(row)
        q, p = divmod(phys, 128)
        if info is not None:
            w2r[p, q * D:(q + 1) * D] = w2[info['cdw']]
            cdw = info['cdw']
            bconv[p, q] = b3[cdw] if cdw < HD // 2 else b5[cdw - HD // 2]
    w2r = w2r.astype(BF16)
    b2rep = np.tile(b2[None, :], (128, 1)).astype(np.float32)

    return dict(w1f=w1f, invn=invn_sb, b1n=b1n_sb, dga=dga, dgb=dgb,
                w2r=w2r, bconv=bconv, b2rep=b2rep)


def _unfold_plan():
    plan = []
    for half in (0, 1):
        base = half * GROWS
        cnt = 0
        for a in range(3):
            for b in range(3):
                for s in range(S[a]):
                    plan.append((half, a, b, s, base + cnt, 20 * S[b]))
                    cnt += 20 * S[b]
    return plan


_UNFOLD_PLAN = _unfold_plan()

_BUILT = None


def _build():
    global _BUILT
    if _BUILT is not None:
        return _BUILT
    import concourse.bacc as bacc
    import concourse.tile as tile
    import concourse.mybir as mybir
    import bass_rust
    from contextlib import ExitStack

    dt = mybir.dt
    AF = mybir.ActivationFunctionType
    OP = mybir.AluOpType

    def view(ap2d, p0, pcnt, off, dims, pstep=1):
        """Arbitrary free-dim view of a [128, F] tile, partitions
        p0, p0+pstep, ... (pcnt of them)."""
        tp = ap2d[p0:p0 + 1, :]
        pitch = ap2d.ap[0][0]
        return bass_rust.AP(tp.tensor, tp.offset + off,
                            [[pitch * pstep, pcnt]]
                            + [[s, c] for s, c in dims])

    nc = bacc.Bacc("TRN2", target_bir_lowering=False, debug=False,
                   enable_asserts=False, num_devices=NCORE)

    x_d = nc.dram_tensor("x_in", [CPC * L, D], dt.float32,
                         kind="ExternalInput").ap()
    w1f_d = nc.dram_tensor("w1f", [128, 21 * 4 * 128], dt.bfloat16,
                           kind="ExternalInput").ap()
    invn_d = nc.dram_tensor("invn", [128, 3 * PMN], dt.bfloat16,
                            kind="ExternalInput").ap()
    b1n_d = nc.dram_tensor("b1n", [128, 3 * PMN], dt.bfloat16,
                           kind="ExternalInput").ap()
    dga_d = nc.dram_tensor("dga", [128, 8 * 9 * 32], dt.bfloat16,
                           kind="ExternalInput").ap()
    dgb_d = nc.dram_tensor("dgb", [128, 8 * 25 * 32], dt.bfloat16,
                           kind="ExternalInput").ap()
    w2r_d = nc.dram_tensor("w2r", [128, NCHUNK * D], dt.bfloat16,
                           kind="ExternalInput").ap()
    bconv_d = nc.dram_tensor("bconv", [128, NCHUNK], dt.float32,
                             kind="ExternalInput").ap()
    b2rep_d = nc.dram_tensor("b2rep", [128, D], dt.float32,
                             kind="ExternalInput").ap()
    out_d = nc.dram_tensor("y_out", [CPC * L, D], dt.float32,
                           kind="ExternalOutput").ap()

    with tile.TileContext(nc) as tc, ExitStack() as ctx:
        dram_pool = ctx.enter_context(
            tc.tile_pool(name="dram", bufs=1, space="DRAM"))
        xbf_d = [dram_pool.tile([L, D], dt.bfloat16, tag=f"xbf{c}",
                                name=f"xbf{c}") for c in range(CPC)]

        consts = ctx.enter_context(tc.tile_pool(name="consts", bufs=1))

        def cload(nm, dram_ap, shape, dtype):
            # consts go on the ACT HWDGE queue so the SP queue is free
            # for the input transposes at startup.
            t = consts.tile(shape, dtype, tag=nm, name=f"c_{nm}")
            nc.scalar.dma_start(t[:, :], dram_ap[:, :])
            return t

        w1f = consts.tile([128, 21 * 4 * 128], dt.bfloat16, tag='w1f',
                          name='c_w1f')
        for k in range(4):
            kw = 21 * 128
            nc.scalar.dma_start(w1f[:, k * kw:(k + 1) * kw],
                                w1f_d[:, k * kw:(k + 1) * kw])
        invn = cload('invn', invn_d, [128, 3 * PMN], dt.bfloat16)
        b1n = cload('b1n', b1n_d, [128, 3 * PMN], dt.bfloat16)
        dga = cload('dga', dga_d, [128, 8 * 9 * 32], dt.bfloat16)
        dgb = cload('dgb', dgb_d, [128, 8 * 25 * 32], dt.bfloat16)
        w2r = cload('w2r', w2r_d, [128, NCHUNK * D], dt.bfloat16)
        bconv = cload('bconv', bconv_d, [128, NCHUNK], dt.float32)
        b2rep = cload('b2rep', b2rep_d, [128, D], dt.float32)

        xt_pool = ctx.enter_context(tc.tile_pool(name="xt", bufs=2))
        imgn_pool = ctx.enter_context(tc.tile_pool(name="imgn", bufs=2))
        x2_pool = ctx.enter_context(tc.tile_pool(name="x2", bufs=2))
        g_pool = ctx.enter_context(tc.tile_pool(name="g", bufs=1))
        osb_pool = ctx.enter_context(tc.tile_pool(name="osb", bufs=1))

        # ---------- input path: cast + transpose for both clips ----------
        # fp32->bf16 cast staged through DRAM, split per (clip, kc) column
        # slice so each transpose (and the kc-outer tconv matmuls) can
        # start as soon as its own slice is cast.
        xts = []
        for clip in range(CPC):
            # contiguous row-halves: few big descriptors, so the SWDGE
            # emission doesn't sit on the critical path like column
            # slices (512B runs x 720) did.
            xb = xbf_d[clip][:, :]
            for rh in range(2):
                o = rh * (L // 2) * D
                src = bass_rust.AP(x_d.tensor,
                                   x_d.offset + clip * L * D + o,
                                   [[D, L // 2], [1, D]])
                dst = bass_rust.AP(xb.tensor, xb.offset + o,
                                   [[D, L // 2], [1, D]])
                nc.gpsimd.dma_start(dst, src)
        for clip in range(CPC):
            xt = xt_pool.tile([128, 4 * L], dt.bfloat16, tag="xt")
            for kc in range(4):
                nc.sync.dma_start(
                    xt[:, kc * L:(kc + 1) * L],
                    xbf_d[clip][:, kc * 128:(kc + 1) * 128],
                    transpose=True)
            xts.append(xt)

        imgns = []
        x2s = []

        def tconv(clip):
            xt = xts[clip]
            imgn = imgn_pool.tile([128, 3 * PMN], dt.bfloat16, tag="imgn")
            # zero the pitch-pad cols (38,39) of every stored row: the
            # contiguous unfold runs sweep them into x2's pad columns.
            nc.vector.memset(view(imgn, 0, 128, PN, [(PNP, 3 * PM), (1, 2)]),
                             0)
            for jh in range(2):
                ps = [tcps.tile([128, 512], dt.float32, tag=f"tc{a}",
                                name=f"tcps{a}")
                      for a in range(3)]
                nmm = {a: sum(1 for (_, _, aa) in TCONV_MMS if aa == a)
                       for a in range(3)}
                # kc-outer: the first 21 matmuls only need xt's kc=0
                # slice, so PE starts as soon as the first transpose
                # lands instead of waiting for the whole xt.
                for k in range(4):
                    cnt = {0: 0, 1: 0, 2: 0}
                    for mi, (s, t, a) in enumerate(TCONV_MMS):
                        cnt[a] += 1
                        if jh == 0:
                            wjd, loc_off, src_off = 19 - t, s * 19 + t, 0
                        else:
                            wjd, loc_off, src_off = 17 + t, s * 19, 19 - t
                        out_ap = view(ps[a], 0, 128, loc_off,
                                      [(19, HOUT), (1, wjd)])
                        rhs = view(xt, 0, 128, k * L + src_off,
                                   [(WOUT, HOUT), (1, wjd)])
                        lhsT = w1f[:, (k * 21 + mi) * 128:
                                   (k * 21 + mi) * 128 + 128]
                        nc.tensor.matmul(out_ap, lhsT, rhs,
                                         start=(cnt[a] == 1 and k == 0),
                                         stop=(cnt[a] == nmm[a]
                                               and k == 3))
                for a in range(3):
                    ps_ap = view(ps[a], 0, 120, 0, [(19, PM), (1, 19)])
                    im_ap = view(imgn, 0, 120, a * PMN + jh * 19,
                                 [(PNP, PM), (1, 19)])
                    iv_ap = view(invn, 0, 120, a * PMN + jh * 19,
                                 [(PNP, PM), (1, 19)])
                    b1_ap = view(b1n, 0, 120, a * PMN + jh * 19,
                                 [(PNP, PM), (1, 19)])
                    nc.vector.tensor_tensor(im_ap, ps_ap, iv_ap, OP.mult)
                    nc.vector.tensor_tensor(im_ap, im_ap, b1_ap, OP.add)
            imgns.append(imgn)

        dma_rr = [0]

        def dma_issue(dst, src, engines=None):
            # alternate the issuing engine: the DMA_DIRECT2D inst occupies
            # its sequencer ~0.6-1.0us regardless of size, so splitting
            # across engines multiplies issue throughput.
            engines = engines or (nc.sync, nc.scalar)
            eng = engines[dma_rr[0] % len(engines)]
            dma_rr[0] += 1
            eng.dma_start(dst, src)

        RUN = (HOUT - 1) * PLW + WOUT  # 796: one contiguous (i,j) sweep

        def unfold(clip):
            # clip 0's DMAs spread over SP+ACT+GpSimd (all idle then, and
            # dw(0) can't start until they land); clip 1's avoid ACT so
            # dw(0)'s gelu ACTIVATEs aren't stuck behind them in the ACT
            # FIFO -- they go to SP+GpSimd instead.
            engines = (nc.sync, nc.scalar, nc.gpsimd) if clip == 0 \
                else (nc.sync, nc.gpsimd)
            imgn = imgns[clip]
            x2a = x2_pool.tile([128, 8 * PLANE], dt.bfloat16, tag="x2a")
            x2b = x2_pool.tile([128, 8 * PLANE], dt.bfloat16, tag="x2b")
            for x2t in (x2a, x2b):
                nc.vector.memset(
                    view(x2t, 0, 128, 0, [(PLANE, 8), (1, 2 * PLW + 2)]), 0)
                nc.vector.memset(
                    view(x2t, 0, 128, 22 * PLW - 2,
                         [(PLANE, 8), (1, 2 * PLW + 2)]), 0)
                # pad rows 980..1023 (chunk 7, partitions 84..127): zero the
                # whole plane so zero-diag matmuls see 0, not NaN garbage.
                # (32-aligned partition base required; unfold rewrites the
                # real rows 960..979 afterwards.)
                nc.vector.memset(x2t[64:128, 7 * PLANE:8 * PLANE], 0)

            # One fan-out DMA per (half, a, b, s): src [nch parts,
            # (t: stride 1, sb), (contig RUN)] -> dst [nch*sb parts,
            # (contig RUN)] (rows ordered (ch, t)), split only at
            # 128-row chunk boundaries. The contiguous runs sweep
            # imgn's zeroed pitch-pad cols into x2's pad columns;
            # the column memset below re-zeroes them afterwards.
            for (half, a, b, s, r0g, n) in _UNFOLD_PLAN:
                sb = S[b]
                x2t = x2a if half == 0 else x2b
                pbase = 40 * b + 20 * half
                r0 = r0g % GROWS
                f = 0
                while f < n:
                    row = r0 + f
                    q, p0 = divmod(row, 128)
                    m = min(n - f, 128 - p0)
                    ch0, t0 = divmod(f, sb)
                    off = a * PMN + s * PNP
                    if t0 != 0 or m < sb:
                        m = min(m, sb - t0)
                        src = view(imgn, pbase + ch0, 1, off + t0,
                                   [(1, m), (1, RUN)])
                    else:
                        m = (m // sb) * sb
                        src = view(imgn, pbase + ch0, m // sb, off,
                                   [(1, sb), (1, RUN)])
                    dst = view(x2t, p0, m, q * PLANE + 2 * PLW + 2,
                               [(1, RUN)])
                    dma_issue(dst, src, engines)
                    f += m
            for x2t in (x2a, x2b):
                # on GpSimd, not DVE: this memset waits on all unfold
                # DMAs, and on the DVE FIFO it would block the next
                # clip's tconv normalize (delaying the PSUM handover
                # from tcps to dwps).
                nc.gpsimd.memset(
                    view(x2t, 0, 128, 2 * PLW + 38,
                         [(PLANE, 8), (PLW, 20), (1, 4)]), 0)
            x2s.append((x2a, x2b))

        with tc.tile_pool(name="tcps", bufs=2, space="PSUM") as tcps:
            for clip in range(CPC):
                tconv(clip)
                unfold(clip)

        for clip in range(CPC):
            x2a, x2b = x2s[clip]
            # ---------- dwconv ----------
            g = g_pool.tile([128, NCHUNK * L + GPAD], dt.bfloat16, tag="g")
            nc.vector.memset(g[:, NCHUNK * L:NCHUNK * L + GPAD], 0)
            with tc.tile_pool(name="dwps", bufs=2, space="PSUM") as dps:
                for grp in range(2):
                    x2t = x2a if grp == 0 else x2b
                    dg = dga if grp == 0 else dgb
                    nuv = 9 if grp == 0 else 25
                    ko = 1 if grp == 0 else 2
                    uvs = [(du, dv) for du in range(-ko, ko + 1)
                           for dv in range(-ko, ko + 1)]
                    for pg in range(2):
                        for jh in range(2):
                            ps = dps.tile([128, 4 * 512], dt.float32,
                                          tag="dw")
                            for uvi, (du, dv) in enumerate(uvs):
                                for ql in range(4):
                                    q = 4 * pg + ql
                                    for r in range(4):
                                        c = (r + q + 8 * grp) % 4
                                        lhsT = dg[32 * r:32 * r + 32,
                                                  (q * nuv + uvi) * 32:
                                                  (q * nuv + uvi + 1) * 32]
                                        rhs = view(
                                            x2t, 32 * r, 32,
                                            q * PLANE + (2 + du) * PLW
                                            + 2 + dv + jh * 18,
                                            [(PLW, HOUT), (1, 18)])
                                        out = view(ps, 32 * c, 32, ql * 512,
                                                   [(18, HOUT), (1, 18)])
                                        nc.tensor.matmul(
                                            out, lhsT, rhs,
                                            start=(uvi == 0),
                                            stop=(uvi == nuv - 1),
                                            tile_position=(32 * r, 32 * c))
                            for ql in range(4):
                                gq = 8 * grp + 4 * pg + ql
                                g_ap = view(g, 0, 128, gq * L + jh * 18,
                                            [(WOUT, HOUT), (1, 18)])
                                ps_ap = view(ps, 0, 128, ql * 512,
                                             [(18, HOUT), (1, 18)])
                                nc.scalar.activation(
                                    g_ap, ps_ap, AF.Gelu,
                                    bias=bconv[:, gq:gq + 1], scale=1.0)

            # ---------- mm2 ----------
            with tc.tile_pool(name="mmps", bufs=2, space="PSUM") as mps:
                osb = osb_pool.tile([128, 6 * D], dt.float32, tag="osb")
                for mt in range(6):
                    pso = mps.tile([128, D], dt.float32, tag="mm2")
                    for kc in range(NCHUNK):
                        lhsT = g[:, kc * L + mt * 120:kc * L + mt * 120 + 128]
                        rhs = w2r[:, kc * D:(kc + 1) * D]
                        nc.tensor.matmul(pso[:, :], lhsT, rhs,
                                         start=(kc == 0),
                                         stop=(kc == NCHUNK - 1))
                    nc.vector.tensor_tensor(osb[0:120, mt * D:(mt + 1) * D],
                                            pso[0:120, :],
                                            b2rep[0:120, :], OP.add)
                    if mt in (2, 5):
                        # two DMAs per clip: first half ships while the
                        # second half's matmuls still run. src order
                        # (p, mt, d) matches dst rows mt*120+p via 3-dim AP.
                        mt0 = mt - 2
                        src = view(osb, 0, 120, mt0 * D, [(D, 3), (1, D)])
                        dst = bass_rust.AP(
                            out_d.tensor,
                            out_d.offset + (clip * L + mt0 * 120) * D,
                            [[D, 120], [120 * D, 3], [1, D]])
                        nc.sync.dma_start(dst, src)

    nc.compile()
    _BUILT = nc
    return nc


def kernel(**inputs):
    x = np.asarray(inputs['x'], np.float32)
    consts = build_consts(
        np.asarray(inputs['w1'], np.float32),
        np.asarray(inputs['b1'], np.float32),
        np.asarray(inputs['w3'], np.float32),
        np.asarray(inputs['b3'], np.float32),
        np.asarray(inputs['w5'], np.float32),
        np.asarray(inputs['b5'], np.float32),
        np.asarray(inputs['w2'], np.float32),
        np.asarray(inputs['b2'], np.float32))
    nc = _build()
    from concourse.bass_utils import run_bass_kernel_spmd

    xf = x.reshape(NCLIP, L, D)
    in_maps = []
    for core in range(NCORE):
        m = {k: consts[k] for k in consts}
        m['x_in'] = np.ascontiguousarray(
            xf[core * CPC:(core + 1) * CPC].reshape(CPC * L, D))
        in_maps.append(m)
    res = run_bass_kernel_spmd(nc, in_maps, core_ids=list(range(NCORE)))
    out = np.zeros((NCLIP, L, D), np.float32)
    for core in range(NCORE):
        out[core * CPC:(core + 1) * CPC] = \
            res.results[core]['y_out'].reshape(CPC, L, D)
    return out.reshape(B, T * L, D)
